# revision 10
# baseline (speedup 1.0000x reference)
"""MinGRU (L=2, B=8, S=2048, D=H=1024) Trainium2 Bass kernel.

Sharding: data-parallel over batch B across the 8 NeuronCores (1 sequence
per core); the (L,H,D) weights are replicated.

Per-core dataflow (all shapes per core):
  inputs (host-preprocessed): xT (D,S) bf16, WzT/WhT (L,D,H) bf16,
  biases as (L,128,H/128) fp32 tiles.
  layer l:
    k  = Wz_l @ x          -> PSUM (h-part, s-free), 8 accumulating matmuls
    p  = Wh_l @ x          -> PSUM
    z  = sigmoid(k + bz)          (ACT, PSUM->SBUF)
    c  = sigmoid(-(k + bz))       (ACT, scale=-1, bias=-bz)
    sg = sigmoid(p + bh)          (ACT)
    g  = max(p + bh + 0.5, sg)    (DVE scalar_tensor_tensor)
    v  = z * g                    (DVE)
    h[t] = c[t]*h[t-1] + v[t], h0=0.5   (DVE tensor_tensor_scan, fp32 state)
  layer-1 scan emits bf16 directly into the layer-2 rhs buffer; layer-2
  scan emits fp32 rows DMA'd to DRAM as (H,S); host transposes to (S,H).
"""

import os
import sys

for _p in (
    "/root/.axon_site",
    "/root/.axon_site/_ro/trn_rl_repo",
    "/root/.axon_site/_ro/pypackages",
    "/opt/trn_rl_repo",
    "/opt/pypackages",
):
    if os.path.isdir(_p) and _p not in sys.path:
        sys.path.append(_p)

from contextlib import ExitStack

import ml_dtypes
import numpy as np

import concourse.bacc as bacc
import concourse.bass as bass
import concourse.tile as tile
from concourse import mybir

L, B, S, D, H = 2, 8, 2048, 1024, 1024
P = 128
DT = D // P          # 8 contraction tiles
HT = H // P          # 8 output-channel tiles
SB = 512             # time-block (one PSUM bank of fp32)
NSB = S // SB        # 4

BF16 = mybir.dt.bfloat16
F32 = mybir.dt.float32
AF = mybir.ActivationFunctionType
OP = mybir.AluOpType

LAST_EXEC_NS = None

_BUILT = None


def _build():
    nc = bacc.Bacc("TRN2", target_bir_lowering=False, debug=False)

    xT = nc.dram_tensor("xT", (D, S), BF16, kind="ExternalInput")
    wzT = nc.dram_tensor("wzT", (L, D, H), BF16, kind="ExternalInput")
    whT = nc.dram_tensor("whT", (L, D, H), BF16, kind="ExternalInput")
    # biases pre-tiled on host: [l, p, ht] = b[l, ht*128 + p]
    bz_d = nc.dram_tensor("bz_t", (L, P, HT), F32, kind="ExternalInput")
    bzn_d = nc.dram_tensor("bzn_t", (L, P, HT), F32, kind="ExternalInput")
    bh_d = nc.dram_tensor("bh_t", (L, P, HT), F32, kind="ExternalInput")
    bh05_d = nc.dram_tensor("bh05_t", (L, P, HT), F32, kind="ExternalInput")
    outT = nc.dram_tensor("outT", (H, S), F32, kind="ExternalOutput")

    xT_r = xT.rearrange("(dt p) s -> p dt s", p=P)

    with tile.TileContext(nc) as tc, ExitStack() as ctx:
        persist = ctx.enter_context(tc.tile_pool(name="persist", bufs=1))
        cvpool = ctx.enter_context(tc.tile_pool(name="cv", bufs=3))
        zpool = ctx.enter_context(tc.tile_pool(name="zs", bufs=3))
        # layer-1 output chunks: chain distance between same-ht chunks is
        # HT units in sb-major order, so keep >=HT+2 slots live
        ochunk_pool = ctx.enter_context(tc.tile_pool(name="ochunk", bufs=HT + 2))
        pk_pool = ctx.enter_context(tc.tile_pool(name="pk", bufs=2, space="PSUM"))
        pp_pool = ctx.enter_context(tc.tile_pool(name="pp", bufs=2, space="PSUM"))

        # ---- persistent SBUF state ----
        x_sb = persist.tile([P, DT, S], BF16)       # layer-0 input (xT)
        h1_sb = persist.tile([P, HT, S], BF16)      # layer-0 output = layer-1 rhs
        w_sb = {}
        for l in range(L):
            for nm, dram in (("wz", wzT), ("wh", whT)):
                t = persist.tile([P, DT, H], BF16, name=f"{nm}{l}_sb")
                src = dram[l].rearrange("(dt p) h -> p dt h", p=P)
                for dt_i in range(DT):
                    nc.sync.dma_start(out=t[:, dt_i], in_=src[:, dt_i])
                w_sb[(nm, l)] = t
        bias_sb = {}
        for nm, dram in (("bz", bz_d), ("bzn", bzn_d), ("bh", bh_d), ("bh05", bh05_d)):
            for l in range(L):
                t = persist.tile([P, HT], F32, name=f"{nm}{l}_sb")
                nc.sync.dma_start(out=t, in_=dram[l])
                bias_sb[(nm, l)] = t

        # x loaded in consumption order (sb-major)
        for sb in range(NSB):
            for dt_i in range(DT):
                nc.sync.dma_start(
                    out=x_sb[:, dt_i, sb * SB : (sb + 1) * SB],
                    in_=xT_r[:, dt_i, sb * SB : (sb + 1) * SB],
                )

        def layer(l, rhs_sb, out_writer):
            """rhs_sb: [P, DT, S] bf16 input; out_writer(ht, sb, c, v) emits scan."""
            wz = w_sb[("wz", l)]
            wh = w_sb[("wh", l)]
            bz_t = bias_sb[("bz", l)]
            bzn_t = bias_sb[("bzn", l)]
            bh_t = bias_sb[("bh", l)]
            bh05_t = bias_sb[("bh05", l)]
            for sb in range(NSB):
                s0, s1 = sb * SB, (sb + 1) * SB
                for ht in range(HT):
                    h0, h1 = ht * P, (ht + 1) * P
                    pk = pk_pool.tile([P, SB], F32, name="pk")
                    pp = pp_pool.tile([P, SB], F32, name="pp")
                    for dt_i in range(DT):
                        nc.tensor.matmul(
                            pk,
                            wz[:, dt_i, h0:h1],
                            rhs_sb[:, dt_i, s0:s1],
                            start=(dt_i == 0),
                            stop=(dt_i == DT - 1),
                        )
                    for dt_i in range(DT):
                        nc.tensor.matmul(
                            pp,
                            wh[:, dt_i, h0:h1],
                            rhs_sb[:, dt_i, s0:s1],
                            start=(dt_i == 0),
                            stop=(dt_i == DT - 1),
                        )
                    z = zpool.tile([P, SB], F32, name="z")
                    sg = zpool.tile([P, SB], F32, name="sg")
                    g = zpool.tile([P, SB], F32, name="g")
                    c = cvpool.tile([P, SB], F32, name="c")
                    v = cvpool.tile([P, SB], F32, name="v")
                    nc.scalar.activation(
                        z, pk, AF.Sigmoid, bias=bz_t[:, ht : ht + 1], scale=1.0
                    )
                    nc.scalar.activation(
                        c, pk, AF.Sigmoid, bias=bzn_t[:, ht : ht + 1], scale=-1.0
                    )
                    nc.scalar.activation(
                        sg, pp, AF.Sigmoid, bias=bh_t[:, ht : ht + 1], scale=1.0
                    )
                    # g = (p + (bh+0.5)) max sigmoid(p+bh)
                    nc.vector.scalar_tensor_tensor(
                        g, pp, bh05_t[:, ht : ht + 1], sg, op0=OP.add, op1=OP.max
                    )
                    nc.vector.tensor_mul(v, z, g)
                    out_writer(ht, sb, c, v)

        # layer 0: scan into h1_sb (bf16), chained across s-blocks
        def l0_writer(ht, sb, c, v):
            dst = h1_sb[:, ht, sb * SB : (sb + 1) * SB]
            init = 0.5 if sb == 0 else h1_sb[:, ht, sb * SB - 1 : sb * SB]
            nc.vector.tensor_tensor_scan(dst, c, v, init, op0=OP.mult, op1=OP.add)

        layer(0, x_sb, l0_writer)

        # layer 1: scan into fp32 chunks, DMA out per (ht, sb) chunk
        prev_chunk = {}

        def l1_writer(ht, sb, c, v):
            oc = ochunk_pool.tile([P, SB], F32, name="oc")
            init = 0.5 if sb == 0 else prev_chunk[ht][:, SB - 1 : SB]
            nc.vector.tensor_tensor_scan(oc, c, v, init, op0=OP.mult, op1=OP.add)
            prev_chunk[ht] = oc
            nc.sync.dma_start(
                out=outT[ht * P : (ht + 1) * P, sb * SB : (sb + 1) * SB], in_=oc
            )

        layer(1, h1_sb, l1_writer)

    nc.finalize()
    return nc


class _Runner:
    """Compile the bass module once into a jitted shard_map over 8 cores."""

    def __init__(self):
        import jax
        from jax.experimental.shard_map import shard_map
        from jax.sharding import Mesh, NamedSharding, PartitionSpec

        from concourse import bass2jax, mybir as _mybir

        self.jax = jax
        nc = _build()
        self.nc = nc
        bass2jax.install_neuronx_cc_hook()

        partition_name = (
            nc.partition_id_tensor.name if nc.partition_id_tensor else None
        )
        in_names, out_names, out_avals, zero_shapes = [], [], [], []
        for alloc in nc.m.functions[0].allocations:
            if not isinstance(_mybir.MemoryLocationSet, type) or not isinstance(
                alloc, _mybir.MemoryLocationSet
            ):
                continue
            name = alloc.memorylocations[0].name
            if alloc.kind == "ExternalInput":
                if name != partition_name:
                    in_names.append(name)
            elif alloc.kind == "ExternalOutput":
                shape = tuple(alloc.tensor_shape)
                dtype = _mybir.dt.np(alloc.dtype)
                out_names.append(name)
                out_avals.append(jax.core.ShapedArray(shape, dtype))
                zero_shapes.append((shape, dtype))
        self.in_names = list(in_names)
        self.out_names = out_names
        self.zero_shapes = zero_shapes
        n_params = len(in_names)
        n_outs = len(out_names)
        all_in_names = in_names + out_names
        if partition_name is not None:
            all_in_names.append(partition_name)
        donate = tuple(range(n_params, n_params + n_outs))

        def _body(*args):
            operands = list(args)
            if partition_name is not None:
                operands.append(bass2jax.partition_id_tensor())
            outs = bass2jax._bass_exec_p.bind(
                *operands,
                out_avals=tuple(out_avals),
                in_names=tuple(all_in_names),
                out_names=tuple(out_names),
                lowering_input_output_aliases=(),
                sim_require_finite=True,
                sim_require_nnan=True,
                nc=nc,
            )
            return tuple(outs)

        devices = jax.devices()[:B]
        assert len(devices) == B
        self.mesh = Mesh(np.asarray(devices), ("core",))
        self.sharding = NamedSharding(self.mesh, PartitionSpec("core"))
        in_specs = (PartitionSpec("core"),) * (n_params + n_outs)
        out_specs = (PartitionSpec("core"),) * n_outs
        self.fn = jax.jit(
            shard_map(
                _body,
                mesh=self.mesh,
                in_specs=in_specs,
                out_specs=out_specs,
                check_rep=False,
            ),
            donate_argnums=donate,
            keep_unused=True,
        )

    def _concat_inputs(self, in_maps):
        return [
            np.concatenate([np.asarray(m[name]) for m in in_maps], axis=0)
            for name in self.in_names
        ]

    def _zeros(self):
        return [
            np.zeros((B * s[0], *s[1:]), dt) for (s, dt) in self.zero_shapes
        ]

    def run(self, in_maps):
        out_arrs = self.fn(*self._concat_inputs(in_maps), *self._zeros())
        return [
            {
                name: np.asarray(out_arrs[i]).reshape(B, -1, *out_arrs[i].shape[1:])[c]
                for i, name in enumerate(self.out_names)
            }
            for c in range(B)
        ]

    def bench(self, in_maps, iters=8):
        """Return (est_ns_per_iter, results_of_last)."""
        import time as _time

        jax = self.jax
        dev_in = [
            jax.device_put(a, self.sharding) for a in self._concat_inputs(in_maps)
        ]
        zero_sets = [
            [jax.device_put(z, self.sharding) for z in self._zeros()]
            for _ in range(iters + 1)
        ]
        out = self.fn(*dev_in, *zero_sets[0])  # warmup
        jax.block_until_ready(out)
        t0 = _time.perf_counter()
        for i in range(iters):
            out = self.fn(*dev_in, *zero_sets[i + 1])
        jax.block_until_ready(out)
        t1 = _time.perf_counter()
        est_ns = (t1 - t0) / iters * 1e9
        results = [
            {
                name: np.asarray(out[i]).reshape(B, -1, *out[i].shape[1:])[c]
                for i, name in enumerate(self.out_names)
            }
            for c in range(B)
        ]
        return est_ns, results


_RUNNER = None


def _get_runner():
    global _RUNNER
    if _RUNNER is None:
        _RUNNER = _Runner()
    return _RUNNER


def kernel(x, Wz, bz, Wh, bh, _bench_iters=0):
    global LAST_EXEC_NS
    x = np.asarray(x, dtype=np.float32)
    Wz = np.asarray(Wz, dtype=np.float32)
    bz = np.asarray(bz, dtype=np.float32)
    Wh = np.asarray(Wh, dtype=np.float32)
    bh = np.asarray(bh, dtype=np.float32)

    bf = ml_dtypes.bfloat16
    xT = np.ascontiguousarray(x.transpose(0, 2, 1)).astype(bf)        # (B, D, S)
    wzT = np.ascontiguousarray(Wz.transpose(0, 2, 1)).astype(bf)      # (L, D, H)
    whT = np.ascontiguousarray(Wh.transpose(0, 2, 1)).astype(bf)

    def tile_bias(b):  # (L, H) -> (L, P, HT) with [l, p, ht] = b[l, ht*P + p]
        return np.ascontiguousarray(
            b.reshape(L, HT, P).transpose(0, 2, 1)
        ).astype(np.float32)

    bz_t = tile_bias(bz)
    bzn_t = tile_bias(-bz)
    bh_t = tile_bias(bh)
    bh05_t = tile_bias(bh + 0.5)

    runner = _get_runner()
    in_maps = [
        {
            "xT": xT[b],
            "wzT": wzT,
            "whT": whT,
            "bz_t": bz_t,
            "bzn_t": bzn_t,
            "bh_t": bh_t,
            "bh05_t": bh05_t,
        }
        for b in range(B)
    ]
    if _bench_iters:
        LAST_EXEC_NS, results = runner.bench(in_maps, iters=_bench_iters)
    else:
        results = runner.run(in_maps)
    out = np.stack([results[b]["outT"].T for b in range(B)], axis=0)
    return np.ascontiguousarray(out.astype(np.float32))
